# revision 24
# baseline (speedup 1.0000x reference)
"""Trainium2 Bass kernel for nn_C3S_RegularLoss.

reference:
    xr = x.reshape(B, P, D); xn = xr / ||xr||_2(axis=-1)
    s = mean_b(xn)                     # (P, D)
    corr = s @ s.T                     # (P, P)
    loss = (sum(corr) - 3*trace(corr) + 2P) / 2 * gamma

Reformulated without the corr matrix, with S = sum_b xn (sum, not mean):
    sum(corr)   = ||sum_p S_p||^2 / B^2
    trace(corr) = sum_p ||S_p||^2 / B^2
    loss = ((A - 3*B2) / B^2 + 2P) / 2 * gamma,  A=||t||^2, t=sum_p S_p

Sharding: data-parallel over the batch dim, 8 cores x 1024 rows.

Per core (v2 structure):
  - stream x as 32 per-part chunks (128 rows x 2048 cols, SWDGE DMA with
    fp32->bf16 cast in flight; HBM read is the 94us roofline)
  - sum-of-squares per row: parts 0/1 on ACT (Square+accum), parts 2/3 on
    DVE (fused tensor_tensor_reduce; bf16 2-tensor op uses 2X_1PORT which
    does NOT lock GpSimd out of the SWDGE descriptor rings) -- keeps both
    engines ~45% busy instead of ACT at 100%, so the endgame chain is short
  - S_p = sum_b x_b,p / ||x_b,p|| via PE: stationary r=1/norm [128,1] per
    part, accumulated in PSUM over all 8 tiles (part p at partition 32p)
  - one tiny warmup AllGather at stream start = rank barrier + TOPSP warm
  - final: evac S -> SBUF (junk rows), one strided-AP DMA of rows
    {0,32,64,96} -> cc_in (4,2048), ONE AllGather (floor ~5us, vs ~14us
    measured AllReduce), then reduce the 32 gathered rows with a single
    [32,5]-mask matmul whose 5th column yields t for free, ACT
    square+accum over [5,2048] PSUM, a (-3,-3,-3,-3,1) combine matmul
    -> A - 3*B2, two DVE scalar ops, out.
"""

import os
import sys

sys.path.insert(0, "/opt/trn_rl_repo")
os.environ.setdefault("MYCRO_LOCAL_CACHE", "1")

import numpy as np

B, F = 8192, 8192
NPARTS = 4
D = F // NPARTS                 # 2048
NCORES = 8
B_CORE = B // NCORES            # 1024
TILE_P = 128
NTILES = B_CORE // TILE_P       # 8
MM_N = 512                      # moving free dim per matmul (PSUM bank)
NCHUNK = D // MM_N              # 4
NROWS = NPARTS * NCORES         # 32 gathered rows

_cache = {}


def _build(ncores=NCORES, warm="AllGather", main="AllGather",
           strided_dma=True, dve_parts=2, dve_mode="ttr_bcast"):
    import concourse.bass as bass  # noqa: F401
    import concourse.mybir as mybir
    from concourse import bacc, tile

    f32 = mybir.dt.float32
    bf16 = mybir.dt.bfloat16
    Act = mybir.ActivationFunctionType
    Alu = mybir.AluOpType

    nc = bacc.Bacc("TRN2", num_devices=ncores, debug=False)
    x_t = nc.dram_tensor("x", [B_CORE, F], f32, kind="ExternalInput")
    g_t = nc.dram_tensor("gamma", [1, 1], f32, kind="ExternalInput")
    out_t = nc.dram_tensor("out", [1, 1], f32, kind="ExternalOutput")

    rg = [list(range(ncores))]
    # rows of the post-collective gathered matrix
    grows = NROWS if main == "AllGather" else NPARTS

    with tile.TileContext(nc) as tc:
        with tc.tile_pool(name="xp", bufs=10) as xp, \
             tc.tile_pool(name="scratch", bufs=2) as scp, \
             tc.tile_pool(name="small", bufs=3) as stp, \
             tc.tile_pool(name="tail", bufs=1) as tlp, \
             tc.tile_pool(name="ps", bufs=1, space="PSUM") as psp, \
             tc.tile_pool(name="dram", bufs=1, space="DRAM") as dram:

            # ---- DRAM collective buffers ----
            w_in = dram.tile([1, 32], f32)
            w_out = dram.tile([ncores, 32], f32)
            cc_in = dram.tile([NPARTS, D], f32)
            ag_out = dram.tile([grows, D], f32)

            # ---- constants (hidden under stream start) ----
            g_sb = tlp.tile([1, 1], f32, tag="g_sb")
            nc.sync.dma_start(g_sb[:], g_t[:])
            wtile = tlp.tile([1, 32], f32, tag="wtile")
            nc.vector.memset(wtile[:], 0.0)
            nc.sync.dma_start(w_in[:], wtile[:])

            # masks [32, 5] bf16: col p selects gathered rows with
            # (row mod 4)==p, col 4 is all-ones (computes t = sum_p S_p)
            # (partition-sliced memsets at base>0 mislower; build the
            # constant patterns from iota + compares instead)
            i32 = mybir.dt.int32
            idx32 = tlp.tile([grows, 1], i32, tag="idx32")
            nc.gpsimd.iota(idx32[:], pattern=[[0, 1]], base=0,
                           channel_multiplier=1)
            idx4 = tlp.tile([grows, 1], i32, tag="idx4")
            nc.vector.tensor_scalar(out=idx4[:], in0=idx32[:],
                                    scalar1=3, scalar2=None,
                                    op0=Alu.bitwise_and)
            masks = tlp.tile([grows, NPARTS + 1], bf16, tag="masks")
            for p in range(NPARTS):
                nc.vector.tensor_scalar(
                    out=masks[:, p:p + 1], in0=idx4[:],
                    scalar1=p, scalar2=None, op0=Alu.is_equal)
            nc.vector.memset(masks[:, NPARTS:NPARTS + 1], 1.0)
            # combine vector: out = -3*B2 + A  (rows 0..3 are ||S_p||^2
            # partials, row 4 is ||t||^2): (idx<4) * -4 + 1
            idx5 = tlp.tile([NPARTS + 1, 1], i32, tag="idx5")
            nc.gpsimd.iota(idx5[:], pattern=[[0, 1]], base=0,
                           channel_multiplier=1)
            comb = tlp.tile([NPARTS + 1, 1], f32, tag="comb")
            nc.vector.tensor_scalar(out=comb[:], in0=idx5[:],
                                    scalar1=4, scalar2=None, op0=Alu.is_lt)
            nc.vector.tensor_scalar(out=comb[:], in0=comb[:],
                                    scalar1=-4.0, scalar2=1.0,
                                    op0=Alu.mult, op1=Alu.add)

            # PSUM: S accumulator (part p at partition 32p), 4 banks.
            S = psp.tile([TILE_P, D], f32, tag="accS")

            # DVE reduce scratch: broadcast dummy (never read)
            dummy = tlp.tile([TILE_P, 1], bf16, tag="dummy")

            warm = None
            for i in range(NTILES):
                rows = x_t[i * TILE_P:(i + 1) * TILE_P, :]
                xt = []
                for p in range(NPARTS):
                    xtp = xp.tile([TILE_P, D], bf16, tag=f"xt{p}")
                    # SWDGE DMA casts fp32 -> bf16 in flight (PE wants
                    # bf16; precision headroom is ~1e3x)
                    nc.gpsimd.dma_start(xtp[:], rows[:, p * D:(p + 1) * D])
                    xt.append(xtp)

                if i == 0 and warm is not None:
                    # warmup collective: pure rank barrier + TOPSP warm,
                    # placed after tile 0's DMA issues so descriptor
                    # generation for the x stream isn't delayed
                    if warm == "AllGather":
                        nc.gpsimd.collective_compute(
                            "AllGather", Alu.bypass, replica_groups=rg,
                            ins=[w_in.opt()], outs=[w_out.opt()])
                    else:
                        nc.gpsimd.collective_compute(
                            "AllReduce", Alu.add, replica_groups=rg,
                            ins=[w_in.opt()], outs=[w_out[0:1, :].opt()])

                # per-row sum of squares: ACT parts then DVE parts
                nact = NPARTS - dve_parts
                ss_a = stp.tile([TILE_P, max(nact, 1)], f32, tag="ss_a")
                ss_d = stp.tile([TILE_P, max(dve_parts, 1)], f32,
                                tag="ss_d")
                sqa = scp.tile([TILE_P, D], bf16, tag="sqa")
                for p in range(nact):
                    nc.scalar.activation(
                        sqa[:], xt[p][:], Act.Square,
                        accum_out=ss_a[:, p:p + 1])
                for p in range(nact, NPARTS):
                    acc = ss_d[:, p - nact:p - nact + 1]
                    if dve_mode == "ttr_bcast":
                        nc.vector.tensor_tensor_reduce(
                            dummy[:].broadcast_to((TILE_P, D)),
                            xt[p][:], xt[p][:], scale=1.0, scalar=0.0,
                            op0=Alu.mult, op1=Alu.add, accum_out=acc)
                    elif dve_mode == "ttr_real":
                        sqd = scp.tile([TILE_P, D], bf16, tag="sqd")
                        nc.vector.tensor_tensor_reduce(
                            sqd[:], xt[p][:], xt[p][:], scale=1.0,
                            scalar=0.0, op0=Alu.mult, op1=Alu.add,
                            accum_out=acc)
                    else:  # two_op
                        sqd = scp.tile([TILE_P, D], bf16, tag="sqd")
                        nc.vector.tensor_mul(sqd[:], xt[p][:], xt[p][:])
                        nc.vector.tensor_reduce(
                            acc, sqd[:], axis=mybir.AxisListType.X,
                            op=Alu.add)

                # norm -> reciprocal -> bf16, per engine-group so ACT
                # parts' matmuls never wait on the last DVE part's DMA
                norm_a = stp.tile([TILE_P, max(nact, 1)], f32,
                                  tag="norm_a")
                norm_d = stp.tile([TILE_P, max(dve_parts, 1)], f32,
                                  tag="norm_d")
                r_a = stp.tile([TILE_P, max(nact, 1)], f32, tag="r_a")
                r_d = stp.tile([TILE_P, max(dve_parts, 1)], f32,
                               tag="r_d")
                rb_a = stp.tile([TILE_P, max(nact, 1)], bf16, tag="rb_a")
                rb_d = stp.tile([TILE_P, max(dve_parts, 1)], bf16,
                                tag="rb_d")
                if nact:
                    nc.scalar.sqrt(norm_a[:], ss_a[:])
                    nc.vector.reciprocal(r_a[:], norm_a[:])
                    nc.vector.tensor_copy(rb_a[:], r_a[:])
                if dve_parts:
                    nc.scalar.sqrt(norm_d[:], ss_d[:])
                    nc.vector.reciprocal(r_d[:], norm_d[:])
                    nc.vector.tensor_copy(rb_d[:], r_d[:])

                rb = {p: (rb_a[:, p:p + 1] if p < nact
                          else rb_d[:, p - nact:p - nact + 1])
                      for p in range(NPARTS)}
                for p in range(NPARTS):
                    for j in range(NCHUNK):
                        nc.tensor.matmul(
                            S[32 * p:32 * p + 1, j * MM_N:(j + 1) * MM_N],
                            lhsT=rb[p],
                            rhs=xt[p][:, j * MM_N:(j + 1) * MM_N],
                            start=(i == 0),
                            stop=(i == NTILES - 1),
                            tile_position=(0, 32 * p))

            # ---- endgame ----
            # evac PSUM -> SBUF full-width (junk rows harmless), split
            # across ACT and DVE so the two halves run in parallel
            s_sb = tlp.tile([TILE_P, D], f32, tag="s_sb")
            nc.scalar.copy(s_sb[:, :D // 2], S[:, :D // 2])
            nc.vector.tensor_copy(s_sb[:, D // 2:], S[:, D // 2:])

            if strided_dma:
                # one strided-AP DMA gathers rows {0,32,64,96}
                nc.sync.dma_start(cc_in[:], s_sb[0:TILE_P:32, :])
            else:
                for p in range(NPARTS):
                    eng = nc.sync if p % 2 == 0 else nc.scalar
                    eng.dma_start(cc_in[p:p + 1, :],
                                  s_sb[32 * p:32 * p + 1, :])

            if main == "AllGather":
                nc.gpsimd.collective_compute(
                    "AllGather", Alu.bypass, replica_groups=rg,
                    ins=[cc_in.opt()], outs=[ag_out.opt()])
            else:
                nc.gpsimd.collective_compute(
                    "AllReduce", Alu.add, replica_groups=rg,
                    ins=[cc_in.opt()], outs=[ag_out.opt()])

            # reload gathered rows as bf16 (cast in DMA)
            ag_sb = tlp.tile([grows, D], bf16, tag="ag_sb")
            nc.gpsimd.dma_start(ag_sb[:], ag_out[:])

            # t5[p, :] = S_global_p (p<4), t5[4, :] = t = sum_p S_global_p
            t5 = psp.tile([NPARTS + 1, D], f32, tag="accT")
            for j in range(NCHUNK):
                nc.tensor.matmul(
                    t5[:, j * MM_N:(j + 1) * MM_N],
                    lhsT=masks[:],
                    rhs=ag_sb[:, j * MM_N:(j + 1) * MM_N],
                    start=True, stop=True, tile_position=(0, 0))

            # row-wise ||.||^2 of the 5 rows (reads PSUM directly)
            sq5 = tlp.tile([NPARTS + 1, D], bf16, tag="sq5")
            acc5 = tlp.tile([NPARTS + 1, 1], f32, tag="acc5")
            nc.scalar.activation(sq5[:], t5[:], Act.Square,
                                 accum_out=acc5[:])

            # A - 3*B2 in one tiny fp32 matmul (reuses S's PSUM banks)
            ba = psp.tile([1, 1], f32, tag="accS")
            nc.tensor.matmul(ba[:], lhsT=comb[:], rhs=acc5[:],
                             start=True, stop=True)

            # loss = ((A - 3*B2) / B^2 + 2P) / 2 * gamma
            l0 = tlp.tile([1, 1], f32, tag="l0")
            nc.vector.tensor_scalar(
                out=l0[:], in0=ba[:],
                scalar1=1.0 / (2.0 * float(B) * float(B)),
                scalar2=float(NPARTS),
                op0=Alu.mult, op1=Alu.add)
            loss = tlp.tile([1, 1], f32, tag="loss")
            nc.vector.tensor_mul(loss[:], l0[:], g_sb[:])
            nc.sync.dma_start(out_t[:], loss[:])

    nc.compile()
    return nc


def _get_nc():
    if "nc" not in _cache:
        warm = os.environ.get("C3S_WARM", "AllGather")
        main = os.environ.get("C3S_MAIN", "AllGather")
        if warm == "None":
            warm = None
        _cache["nc"] = _build(
            warm=warm, main=main,
            strided_dma=os.environ.get("C3S_STRIDED", "1") == "1",
            dve_parts=int(os.environ.get("C3S_DVE_PARTS", "2")),
            dve_mode=os.environ.get("C3S_DVE_MODE", "two_op"))
    return _cache["nc"]


def kernel(x, gamma, **run_kwargs):
    from concourse import bass_utils

    x = np.ascontiguousarray(np.asarray(x, dtype=np.float32))
    gamma = np.asarray(gamma, dtype=np.float32).reshape(1, 1)
    assert x.shape == (B, F), x.shape

    nc = _get_nc()
    in_maps = [
        {"x": x[c * B_CORE:(c + 1) * B_CORE], "gamma": gamma}
        for c in range(NCORES)
    ]
    res = bass_utils.run_bass_kernel_spmd(
        nc, in_maps, core_ids=list(range(NCORES)), **run_kwargs)
    out = np.asarray(res.results[0]["out"], dtype=np.float32).reshape(1)
    if run_kwargs.get("trace"):
        _cache["last_results"] = res
    return out


# revision 30
# speedup vs baseline: 1.0919x; 1.0919x over previous
"""Trainium2 Bass kernel for nn_C3S_RegularLoss.

reference:
    xr = x.reshape(B, P, D); xn = xr / ||xr||_2(axis=-1)
    s = mean_b(xn)                     # (P, D)
    corr = s @ s.T                     # (P, P)
    loss = (sum(corr) - 3*trace(corr) + 2P) / 2 * gamma

Reformulated without the corr matrix, with S = sum_b xn (sum, not mean):
    sum(corr)   = ||sum_p S_p||^2 / B^2
    trace(corr) = sum_p ||S_p||^2 / B^2
    loss = ((A - 3*B2) / B^2 + 2P) / 2 * gamma,  A=||t||^2, t=sum_p S_p

Sharding: data-parallel over the batch dim, 8 cores x 1024 rows.

Per core (v2 structure):
  - stream x as 32 per-part chunks (128 rows x 2048 cols, SWDGE DMA with
    fp32->bf16 cast in flight; HBM read is the 94us roofline)
  - sum-of-squares per row: parts 0/1 on ACT (Square+accum), parts 2/3 on
    DVE (fused tensor_tensor_reduce; bf16 2-tensor op uses 2X_1PORT which
    does NOT lock GpSimd out of the SWDGE descriptor rings) -- keeps both
    engines ~45% busy instead of ACT at 100%, so the endgame chain is short
  - S_p = sum_b x_b,p / ||x_b,p|| via PE: stationary r=1/norm [128,1] per
    part, accumulated in PSUM over all 8 tiles (part p at partition 32p)
  - one tiny warmup AllGather at stream start = rank barrier + TOPSP warm
  - final: evac S -> SBUF (junk rows), one strided-AP DMA of rows
    {0,32,64,96} -> cc_in (4,2048), ONE AllGather (floor ~5us, vs ~14us
    measured AllReduce), then reduce the 32 gathered rows with a single
    [32,5]-mask matmul whose 5th column yields t for free, ACT
    square+accum over [5,2048] PSUM, a (-3,-3,-3,-3,1) combine matmul
    -> A - 3*B2, two DVE scalar ops, out.
"""

import os
import sys

sys.path.insert(0, "/opt/trn_rl_repo")
os.environ.setdefault("MYCRO_LOCAL_CACHE", "1")

import numpy as np

B, F = 8192, 8192
NPARTS = 4
D = F // NPARTS                 # 2048
NCORES = 8
B_CORE = B // NCORES            # 1024
TILE_P = 128
NTILES = B_CORE // TILE_P       # 8
MM_N = 512                      # moving free dim per matmul (PSUM bank)
NCHUNK = D // MM_N              # 4
NROWS = NPARTS * NCORES         # 32 gathered rows

_cache = {}


def _build(ncores=NCORES, warm="AllGather", main="AllGather",
           strided_dma=True, dve_parts=2, dve_mode="ttr_bcast"):
    import concourse.bass as bass  # noqa: F401
    import concourse.mybir as mybir
    from concourse import bacc, tile

    f32 = mybir.dt.float32
    bf16 = mybir.dt.bfloat16
    Act = mybir.ActivationFunctionType
    Alu = mybir.AluOpType

    nc = bacc.Bacc("TRN2", num_devices=ncores, debug=False)
    x_t = nc.dram_tensor("x", [B_CORE, F], f32, kind="ExternalInput")
    g_t = nc.dram_tensor("gamma", [1, 1], f32, kind="ExternalInput")
    out_t = nc.dram_tensor("out", [1, 1], f32, kind="ExternalOutput")

    rg = [list(range(ncores))]
    # rows of the post-collective gathered matrix
    grows = NROWS if main == "AllGather" else NPARTS

    with tile.TileContext(nc) as tc:
        with tc.tile_pool(name="xp", bufs=10) as xp, \
             tc.tile_pool(name="scratch", bufs=2) as scp, \
             tc.tile_pool(name="small", bufs=3) as stp, \
             tc.tile_pool(name="tail", bufs=1) as tlp, \
             tc.tile_pool(name="ps", bufs=1, space="PSUM") as psp, \
             tc.tile_pool(name="dram", bufs=1, space="DRAM") as dram:

            # ---- DRAM collective buffers ----
            # warmup buffers have the SAME shape as the real collective:
            # the warm-path benefit is payload-shape-specific (a tiny
            # warmup leaves the real-size collective cold, ~35us vs ~14)
            w_in = dram.tile([NPARTS, D], f32)
            w_out = dram.tile([grows, D], f32)
            cc_in = dram.tile([NPARTS, D], f32)
            ag_out = dram.tile([grows, D], f32)

            # ---- constants (hidden under stream start) ----
            g_sb = tlp.tile([1, 1], f32, tag="g_sb")
            nc.sync.dma_start(g_sb[:], g_t[:])

            # masks [32, 5] bf16: col p selects gathered rows with
            # (row mod 4)==p, col 4 is all-ones (computes t = sum_p S_p)
            # (partition-sliced memsets at base>0 mislower; build the
            # constant patterns from iota + compares instead)
            i32 = mybir.dt.int32
            idx32 = tlp.tile([grows, 1], i32, tag="idx32")
            nc.gpsimd.iota(idx32[:], pattern=[[0, 1]], base=0,
                           channel_multiplier=1)
            idx4 = tlp.tile([grows, 1], i32, tag="idx4")
            nc.vector.tensor_scalar(out=idx4[:], in0=idx32[:],
                                    scalar1=3, scalar2=None,
                                    op0=Alu.bitwise_and)
            masks = tlp.tile([grows, NPARTS + 1], bf16, tag="masks")
            for p in range(NPARTS):
                nc.vector.tensor_scalar(
                    out=masks[:, p:p + 1], in0=idx4[:],
                    scalar1=p, scalar2=None, op0=Alu.is_equal)
            nc.vector.memset(masks[:, NPARTS:NPARTS + 1], 1.0)
            # combine vector: out = -3*B2 + A  (rows 0..3 are ||S_p||^2
            # partials, row 4 is ||t||^2): (idx<4) * -4 + 1
            idx5 = tlp.tile([NPARTS + 1, 1], i32, tag="idx5")
            nc.gpsimd.iota(idx5[:], pattern=[[0, 1]], base=0,
                           channel_multiplier=1)
            comb = tlp.tile([NPARTS + 1, 1], f32, tag="comb")
            nc.vector.tensor_scalar(out=comb[:], in0=idx5[:],
                                    scalar1=4, scalar2=None, op0=Alu.is_lt)
            nc.vector.tensor_scalar(out=comb[:], in0=comb[:],
                                    scalar1=-4.0, scalar2=1.0,
                                    op0=Alu.mult, op1=Alu.add)

            # PSUM: S accumulator (part p at partition 32p), 4 banks.
            S = psp.tile([TILE_P, D], f32, tag="accS")

            # DVE reduce scratch: broadcast dummy (never read)
            dummy = tlp.tile([TILE_P, 1], bf16, tag="dummy")

            for i in range(NTILES):
                last = i == NTILES - 1
                rows = x_t[i * TILE_P:(i + 1) * TILE_P, :]
                # SWDGE DMA casts fp32 -> bf16 in flight (PE wants bf16;
                # precision headroom is ~1e3x). Whole-tile transfers keep
                # 32KB-contiguous HBM reads (per-part 8KB chunks cost
                # ~5-9% HBM efficiency); the last tile is split per part
                # so its compute chain starts at the first part boundary.
                xtile = xp.tile([TILE_P, F], bf16, tag="xt")
                if last:
                    for p in range(NPARTS):
                        nc.gpsimd.dma_start(xtile[:, p * D:(p + 1) * D],
                                            rows[:, p * D:(p + 1) * D])
                else:
                    nc.gpsimd.dma_start(xtile[:], rows)
                xt = [xtile[:, p * D:(p + 1) * D] for p in range(NPARTS)]

                if i == 0 and warm is not None:
                    # warmup collective: rank barrier + warms the
                    # same-shape collective path for the real one at the
                    # end. Reads uninitialized DRAM (result unused) so
                    # its doorbell has no input dependency. Placed after
                    # tile 0's DMA issues so descriptor generation for
                    # the x stream isn't delayed.
                    if warm == "AllGather":
                        nc.gpsimd.collective_compute(
                            "AllGather", Alu.bypass, replica_groups=rg,
                            ins=[w_in.opt()], outs=[w_out.opt()])
                    else:
                        nc.gpsimd.collective_compute(
                            "AllReduce", Alu.add, replica_groups=rg,
                            ins=[w_in.opt()], outs=[w_out[0:NPARTS, :].opt()])

                # per-row sum of squares: ACT parts then DVE parts
                nact = NPARTS - dve_parts
                ss_a = stp.tile([TILE_P, max(nact, 1)], f32, tag="ss_a")
                ss_d = stp.tile([TILE_P, max(dve_parts, 1)], f32,
                                tag="ss_d")
                sqa = scp.tile([TILE_P, D], bf16, tag="sqa")
                for p in range(nact):
                    nc.scalar.activation(
                        sqa[:], xt[p], Act.Square,
                        accum_out=ss_a[:, p:p + 1])
                for p in range(nact, NPARTS):
                    acc = ss_d[:, p - nact:p - nact + 1]
                    if dve_mode == "ttr_bcast":
                        nc.vector.tensor_tensor_reduce(
                            dummy[:].broadcast_to((TILE_P, D)),
                            xt[p], xt[p], scale=1.0, scalar=0.0,
                            op0=Alu.mult, op1=Alu.add, accum_out=acc)
                    elif dve_mode == "ttr_real":
                        sqd = scp.tile([TILE_P, D], bf16, tag="sqd")
                        nc.vector.tensor_tensor_reduce(
                            sqd[:], xt[p], xt[p], scale=1.0,
                            scalar=0.0, op0=Alu.mult, op1=Alu.add,
                            accum_out=acc)
                    else:  # two_op
                        sqd = scp.tile([TILE_P, D], bf16, tag="sqd")
                        nc.vector.tensor_mul(sqd[:], xt[p], xt[p])
                        nc.vector.tensor_reduce(
                            acc, sqd[:], axis=mybir.AxisListType.X,
                            op=Alu.add)

                # norm -> reciprocal -> bf16, per engine-group so ACT
                # parts' matmuls never wait on the last DVE part's DMA
                norm_a = stp.tile([TILE_P, max(nact, 1)], f32,
                                  tag="norm_a")
                norm_d = stp.tile([TILE_P, max(dve_parts, 1)], f32,
                                  tag="norm_d")
                r_a = stp.tile([TILE_P, max(nact, 1)], f32, tag="r_a")
                r_d = stp.tile([TILE_P, max(dve_parts, 1)], f32,
                               tag="r_d")
                rb_a = stp.tile([TILE_P, max(nact, 1)], bf16, tag="rb_a")
                rb_d = stp.tile([TILE_P, max(dve_parts, 1)], bf16,
                                tag="rb_d")
                if nact:
                    nc.scalar.sqrt(norm_a[:], ss_a[:])
                    nc.vector.reciprocal(r_a[:], norm_a[:])
                    nc.vector.tensor_copy(rb_a[:], r_a[:])
                if dve_parts:
                    nc.scalar.sqrt(norm_d[:], ss_d[:])
                    nc.vector.reciprocal(r_d[:], norm_d[:])
                    nc.vector.tensor_copy(rb_d[:], r_d[:])

                rb = {p: (rb_a[:, p:p + 1] if p < nact
                          else rb_d[:, p - nact:p - nact + 1])
                      for p in range(NPARTS)}
                for p in range(NPARTS):
                    for j in range(NCHUNK):
                        nc.tensor.matmul(
                            S[32 * p:32 * p + 1, j * MM_N:(j + 1) * MM_N],
                            lhsT=rb[p],
                            rhs=xtile[:, p * D + j * MM_N:
                                      p * D + (j + 1) * MM_N],
                            start=(i == 0),
                            stop=(i == NTILES - 1),
                            tile_position=(0, 32 * p))

            # ---- endgame ----
            # evac PSUM -> SBUF full-width (junk rows harmless); two
            # SEPARATE tiles so the ACT and DVE halves truly run in
            # parallel (a shared tile serializes them via dep tracking)
            s_lo = tlp.tile([TILE_P, D // 2], f32, tag="s_lo")
            s_hi = tlp.tile([TILE_P, D // 2], f32, tag="s_hi")
            nc.scalar.copy(s_lo[:], S[:, :D // 2])
            nc.vector.tensor_copy(s_hi[:], S[:, D // 2:])

            if strided_dma:
                # strided-AP DMAs gather rows {0,32,64,96}
                nc.sync.dma_start(cc_in[:, :D // 2], s_lo[0:TILE_P:32, :])
                nc.scalar.dma_start(cc_in[:, D // 2:], s_hi[0:TILE_P:32, :])
            else:
                for p in range(NPARTS):
                    eng = nc.sync if p % 2 == 0 else nc.scalar
                    eng.dma_start(cc_in[p:p + 1, :D // 2],
                                  s_lo[32 * p:32 * p + 1, :])
                    eng.dma_start(cc_in[p:p + 1, D // 2:],
                                  s_hi[32 * p:32 * p + 1, :])

            if main == "AllGather":
                nc.gpsimd.collective_compute(
                    "AllGather", Alu.bypass, replica_groups=rg,
                    ins=[cc_in.opt()], outs=[ag_out.opt()])
            else:
                nc.gpsimd.collective_compute(
                    "AllReduce", Alu.add, replica_groups=rg,
                    ins=[cc_in.opt()], outs=[ag_out.opt()])

            # reload gathered rows as bf16 (cast in DMA)
            ag_sb = tlp.tile([grows, D], bf16, tag="ag_sb")
            nc.gpsimd.dma_start(ag_sb[:], ag_out[:])

            # t5[p, :] = S_global_p (p<4), t5[4, :] = t = sum_p S_global_p
            t5 = psp.tile([NPARTS + 1, D], f32, tag="accT")
            for j in range(NCHUNK):
                nc.tensor.matmul(
                    t5[:, j * MM_N:(j + 1) * MM_N],
                    lhsT=masks[:],
                    rhs=ag_sb[:, j * MM_N:(j + 1) * MM_N],
                    start=True, stop=True, tile_position=(0, 0))

            # row-wise ||.||^2 of the 5 rows (reads PSUM directly)
            sq5 = tlp.tile([NPARTS + 1, D], bf16, tag="sq5")
            acc5 = tlp.tile([NPARTS + 1, 1], f32, tag="acc5")
            nc.scalar.activation(sq5[:], t5[:], Act.Square,
                                 accum_out=acc5[:])

            # A - 3*B2 in one tiny fp32 matmul (reuses S's PSUM banks)
            ba = psp.tile([1, 1], f32, tag="accS")
            nc.tensor.matmul(ba[:], lhsT=comb[:], rhs=acc5[:],
                             start=True, stop=True)

            # loss = ((A - 3*B2) / B^2 + 2P) / 2 * gamma
            l0 = tlp.tile([1, 1], f32, tag="l0")
            nc.vector.tensor_scalar(
                out=l0[:], in0=ba[:],
                scalar1=1.0 / (2.0 * float(B) * float(B)),
                scalar2=float(NPARTS),
                op0=Alu.mult, op1=Alu.add)
            loss = tlp.tile([1, 1], f32, tag="loss")
            nc.vector.tensor_mul(loss[:], l0[:], g_sb[:])
            nc.sync.dma_start(out_t[:], loss[:])

    nc.compile()
    return nc


def _get_nc():
    if "nc" not in _cache:
        warm = os.environ.get("C3S_WARM", "AllGather")
        main = os.environ.get("C3S_MAIN", "AllGather")
        if warm == "None":
            warm = None
        _cache["nc"] = _build(
            warm=warm, main=main,
            strided_dma=os.environ.get("C3S_STRIDED", "1") == "1",
            dve_parts=int(os.environ.get("C3S_DVE_PARTS", "2")),
            dve_mode=os.environ.get("C3S_DVE_MODE", "two_op"))
    return _cache["nc"]


def kernel(x, gamma, **run_kwargs):
    from concourse import bass_utils

    x = np.ascontiguousarray(np.asarray(x, dtype=np.float32))
    gamma = np.asarray(gamma, dtype=np.float32).reshape(1, 1)
    assert x.shape == (B, F), x.shape

    nc = _get_nc()
    in_maps = [
        {"x": x[c * B_CORE:(c + 1) * B_CORE], "gamma": gamma}
        for c in range(NCORES)
    ]
    res = bass_utils.run_bass_kernel_spmd(
        nc, in_maps, core_ids=list(range(NCORES)), **run_kwargs)
    out = np.asarray(res.results[0]["out"], dtype=np.float32).reshape(1)
    if run_kwargs.get("trace"):
        _cache["last_results"] = res
    return out
